# revision 24
# baseline (speedup 1.0000x reference)
"""GATModelVAE (2-layer GAT encoder VAE, eval mode) on 8 Trainium2 NeuronCores.

Strategy: destination-node (graph) parallelism. Nodes are packed into
160 windows of 128 dst nodes (degree-sorted, banded so all 8 cores run an
identical program). Per window, incoming edges live in an ELL (slot-major)
layout: slot j of partition n is the j-th in-edge of window-node n; padded
slots point at a sentinel table row whose att-logit columns are -1e4 so
exp() gives exactly 0. Per-edge source features arrive via dma_gather from
an AllGather-replicated [N, 384] table (xw | a_src | a_dst columns), the
softmax numerator weighting is a single in-place DVE multiply, and the
segment sum over edges is a PSUM-accumulated identity matmul that also
accumulates the softmax denominators in the same instruction. The softmax
normalization (constant per destination node) is applied after aggregation
in the window epilogue.
"""

import sys

sys.path.insert(0, "/opt/trn_rl_repo")

import numpy as np

N = 20000
E0 = 320000
FIN = 256
H1 = 64
H2 = 32
HEADS = 5
NEG = 0.2

NCORE = 8
P = 128
NWIN = 160            # global windows
NB = NWIN // NCORE    # windows (bands) per core: 20
MLOC = NB * P         # node slots per core: 2560
CONTRIB = MLOC        # per-core AG contribution rows
TROWS = NCORE * MLOC + 8   # + locally-written sentinel row (pad to 8)
SENT = NCORE * MLOC   # sentinel table row
QROWS = MLOC // 4     # AG quarter rows (5 bands)
NQ = 4                # table-2 AG quarters
WB = 384              # table row width (elems); fp16: 768B, %256 == 0
SLOT_CAP = 8          # max ELL slots per gather chunk (1024 idx = 64-desc packet cap)
TABLE_FP16 = True     # gathered tables in fp16 (halves gather bytes)

_compiled = None  # (key, nc, meta)
TRACE = False          # set True (e.g. from test.py) to capture an NTFF profile
TRACE_DIR = None       # optional dir for trace artifacts
LAST_RESULTS = None    # BassKernelResults of the most recent run


# ----------------------------------------------------------------------------
# host-side graph preparation
# ----------------------------------------------------------------------------
def _prep_graph(edge_index):
    src = np.concatenate([edge_index[0], np.arange(N, dtype=np.int64)])
    dst = np.concatenate([edge_index[1], np.arange(N, dtype=np.int64)])
    EE = src.shape[0]
    deg = np.bincount(dst, minlength=N)

    order = np.argsort(-deg, kind="stable")      # nodes by degree desc
    pos = np.empty(N, np.int64)
    pos[order] = np.arange(N)
    win = pos // P                               # global window id (0..156)
    slot = pos % P
    core = win % NCORE
    band = win // NCORE

    # slots per band = max degree in band (shared by all 8 cores)
    D_band = np.zeros(NB, np.int64)
    np.maximum.at(D_band, band, deg)
    D_band = np.maximum(D_band, 1)

    trow1 = core * MLOC + band * P + slot        # table-1 row (rank-major AG)

    outrow = band * P + slot                     # per-core output row

    # per-edge ELL coordinates: (core, band, slot of dst, j = rank among dst's edges)
    eorder = np.argsort(dst, kind="stable")
    ds = dst[eorder]
    run_start = np.r_[0, np.flatnonzero(ds[1:] != ds[:-1]) + 1]
    j_in = np.arange(EE) - np.repeat(run_start, np.diff(np.r_[run_start, EE]))
    es, ed = src[eorder], ds

    ec, eb, eslot = core[ed], band[ed], slot[ed]

    # chunk layout (uniform across cores)
    chunks = []  # list of (band, d_c), evenly split under SLOT_CAP
    for k in range(NB):
        d = int(D_band[k])
        n = -(-d // SLOT_CAP)
        base, rem = d // n, d % n
        for i in range(n):
            chunks.append((k, base + 1 if i < rem else base))

    # build per-core wrapped int16 index tensors (one per table layout)
    idx_cols = sum(8 * c for (_, c) in chunks)

    def build_idx(trow):
        esrc_row = trow[es].astype(np.int32)
        idx_all = np.full((NCORE, 16, idx_cols), SENT, np.int16)
        ell = {}
        for k in range(NB):
            a = np.full((NCORE, int(D_band[k]), P), SENT, np.int32)
            m = eb == k
            a[ec[m], j_in[m], eslot[m]] = esrc_row[m]
            ell[k] = a
        joffs = {k: 0 for k in range(NB)}
        col = 0
        for (k, d_c) in chunks:
            j0 = joffs[k]
            blk = ell[k][:, j0 : j0 + d_c, :].reshape(NCORE, -1)
            wrapped = blk.reshape(NCORE, -1, 16).transpose(0, 2, 1)
            idx_all[:, :, col : col + 8 * d_c] = wrapped.astype(np.int16)
            joffs[k] = j0 + d_c
            col += 8 * d_c
        assert col == idx_cols
        return np.tile(idx_all, (1, 8, 1))

    meta = dict(
        chunks=chunks, idx_cols=idx_cols, core=core, band=band, slot=slot,
        outrow=outrow,
    )
    return build_idx(trow1), meta


def _w_aug(W, att_s, att_d, heads, hc):
    fin = W.shape[0]
    Wr = W.reshape(fin, heads, hc)
    ws = np.einsum("fhc,hc->fh", Wr, att_s)
    wd = np.einsum("fhc,hc->fh", Wr, att_d)
    return ws.astype(np.float32), wd.astype(np.float32)


# ----------------------------------------------------------------------------
# device program
# ----------------------------------------------------------------------------
def _build_program(chunks, idx_cols):
    import concourse.bass as bass
    import concourse.bacc as bacc
    import concourse.mybir as mybir
    import concourse.tile as tile
    from concourse import library_config
    from concourse.masks import make_identity

    f32 = mybir.dt.float32
    tdt = mybir.dt.float16 if TABLE_FP16 else f32
    AF = mybir.ActivationFunctionType
    OP = mybir.AluOpType

    nc = bacc.Bacc("TRN2", target_bir_lowering=False, debug=False,
                   num_devices=NCORE, num_swdge_queues=4)

    xT_d = nc.dram_tensor("xT", [FIN, MLOC], f32, kind="ExternalInput").ap()
    w1_d = nc.dram_tensor("w1big", [FIN, WB], f32, kind="ExternalInput").ap()
    w2_d = nc.dram_tensor("w2big", [H1, WB], f32, kind="ExternalInput").ap()
    sent_d = nc.dram_tensor("sent", [1, WB], tdt, kind="ExternalInput").ap()
    idx1_d = nc.dram_tensor("idx1", [P, idx_cols], mybir.dt.int16,
                            kind="ExternalInput").ap()
    b1_d = nc.dram_tensor("b1r", [P, H1], f32, kind="ExternalInput").ap()
    b2_d = nc.dram_tensor("b2r", [P, H2], f32, kind="ExternalInput").ap()
    b3_d = nc.dram_tensor("b3r", [P, H2], f32, kind="ExternalInput").ap()

    mu_d = nc.dram_tensor("mu", [MLOC, H2], f32, kind="ExternalOutput").ap()
    lv_d = nc.dram_tensor("lv", [MLOC, H2], f32, kind="ExternalOutput").ap()

    con1_d = nc.dram_tensor("contrib1", [CONTRIB, WB], tdt).ap()
    con2_d = nc.dram_tensor("contrib2", [CONTRIB, WB], tdt).ap()
    tbl1_d = nc.dram_tensor("tbl1", [TROWS, WB], tdt, addr_space="Shared").ap()
    tbl2_d = nc.dram_tensor("tbl2", [TROWS, WB], tdt, addr_space="Shared").ap()

    rg = [list(range(NCORE))]

    with tile.TileContext(nc) as tc:
        with (
            tc.tile_pool(name="const", bufs=1) as cpool,
            tc.tile_pool(name="resid", bufs=1) as rpool,
            tc.tile_pool(name="io", bufs=3) as iopool,
            tc.tile_pool(name="gat", bufs=6) as gpool,
            tc.tile_pool(name="small", bufs=8) as spool,
            tc.tile_pool(name="psum", bufs=3, space="PSUM") as pspool,
            tc.tile_pool(name="psumT", bufs=2, space="PSUM") as ptpool,
        ):
            nc.gpsimd.load_library(library_config.mlp)

            ident = cpool.tile([P, P], f32)
            make_identity(nc, ident[:])
            ident_t = cpool.tile([P, P], tdt)
            nc.vector.tensor_copy(ident_t[:], ident[:])

            w1_t = cpool.tile([P, 2, WB], f32)
            nc.sync.dma_start(w1_t[:], w1_d[:].rearrange("(k p) n -> p k n", p=P))
            w2_t = cpool.tile([H1, WB], f32)
            nc.sync.dma_start(w2_t[:], w2_d[:])
            sent_t = cpool.tile([1, WB], tdt)
            nc.sync.dma_start(sent_t[:], sent_d[:])
            b1_t = cpool.tile([P, H1], f32)
            nc.sync.dma_start(b1_t[:], b1_d[:])
            b2_t = cpool.tile([P, H2], f32)
            nc.sync.dma_start(b2_t[:], b2_d[:])
            b3_t = cpool.tile([P, H2], f32)
            nc.sync.dma_start(b3_t[:], b3_d[:])

            idx1_t = rpool.tile([P, idx_cols], mybir.dt.int16)
            nc.scalar.dma_start(idx1_t[:], idx1_d[:])
            xt_all = rpool.tile([P, 2, MLOC], f32)
            nc.sync.dma_start(xt_all[:], xT_d[:].rearrange("(k p) n -> p k n", p=P))

            ad1 = rpool.tile([P, NB, 8], f32)
            ad23 = rpool.tile([P, NB, 16], f32)
            h1T = rpool.tile([H1, MLOC], f32)

            # ---------------- pass A: layer-1 table -------------------------
            nc.sync.dma_start(tbl1_d[SENT : SENT + 1, :], sent_t[:])
            nc.sync.dma_start(tbl2_d[SENT : SENT + 1, :], sent_t[:])
            for m in range(NB):
                ps = pspool.tile([P, WB], f32, space="PSUM", tag="xwps")
                for kk in range(2):
                    nc.tensor.matmul(ps[:], xt_all[:, kk, m * P : (m + 1) * P],
                                     w1_t[:, kk, :],
                                     start=(kk == 0), stop=(kk == 1))
                row_t = iopool.tile([P, WB], tdt, tag="rowt")
                nc.vector.tensor_copy(row_t[:], ps[:])
                nc.vector.tensor_copy(ad1[:, m, 0:5], ps[:, 325:330])
                nc.sync.dma_start(con1_d[m * P : (m + 1) * P, :], row_t[:])

            nc.gpsimd.collective_compute(
                "AllGather", mybir.AluOpType.bypass, replica_groups=rg,
                ins=[con1_d[:]], outs=[tbl1_d[0 : NCORE * MLOC, :]],
            )

            # ---------------- pass A: layer-1 windows -----------------------
            col = 0
            cur_band = -1
            band_chunks = {}
            for (k, d_c) in chunks:
                band_chunks.setdefault(k, []).append((col, d_c))
                col += 8 * d_c

            for k in range(NB):
                acc = pspool.tile([P, 336], f32, space="PSUM", tag="acc")
                n_mm = sum(d_c for (_, d_c) in band_chunks[k])
                mm_i = 0
                for ci, (coff, d_c) in enumerate(band_chunks[k]):
                    gt = gpool.tile([P, SLOT_CAP, WB], tdt, tag="gt")
                    nidx = P * d_c
                    nc.gpsimd.dma_gather(
                        gt[:, 0:d_c, :], tbl1_d[:],
                        idx1_t[:, coff : coff + 8 * d_c], nidx, nidx, WB,
                        queue_num=(k * 3 + ci) % 4,
                    )
                    ut = spool.tile([P, SLOT_CAP, 5], f32, tag="ut")
                    nc.vector.tensor_tensor(
                        out=ut[:, 0:d_c, :], in0=gt[:, 0:d_c, 320:325],
                        in1=ad1[:, k, 0:5].unsqueeze(1).to_broadcast([P, d_c, 5]),
                        op=OP.add,
                    )
                    lt = spool.tile([P, SLOT_CAP, 5], f32, tag="lt")
                    nc.scalar.activation(lt[:, 0:d_c, :], ut[:, 0:d_c, :],
                                         AF.Prelu, alpha=NEG)
                    nc.scalar.activation(gt[:, 0:d_c, 320:325], lt[:, 0:d_c, :],
                                         AF.Exp)
                    nc.vector.tensor_tensor(
                        out=gt[:, 0:d_c, 0:320].rearrange(
                            "p d (h c) -> p d h c", h=HEADS),
                        in0=gt[:, 0:d_c, 0:320].rearrange(
                            "p d (h c) -> p d h c", h=HEADS),
                        in1=gt[:, 0:d_c, 320:325].unsqueeze(3).to_broadcast(
                            [P, d_c, 5, H1]),
                        op=OP.mult,
                    )
                    for j in range(d_c):
                        nc.tensor.matmul(acc[:, 0:325], ident_t[:], gt[:, j, 0:325],
                                         start=(mm_i == 0), stop=(mm_i == n_mm - 1))
                        mm_i += 1
                # epilogue: out64 = relu(sum_h raw_h * recip_h + b1)
                den = spool.tile([P, 5], f32, tag="den")
                nc.vector.tensor_scalar(out=den[:], in0=acc[:, 320:325],
                                        scalar1=float(HEADS), scalar2=HEADS * 1e-16,
                                        op0=OP.mult, op1=OP.add)
                rec = spool.tile([P, 5], f32, tag="rec")
                nc.vector.reciprocal(rec[:], den[:])
                tmp = spool.tile([P, H1, HEADS], f32, tag="tmp1")
                nc.vector.tensor_tensor(
                    out=tmp[:].transpose([0, 2, 1]),
                    in0=acc[:, 0:320].rearrange("p (h c) -> p h c", h=HEADS),
                    in1=rec[:].unsqueeze(2).to_broadcast([P, HEADS, H1]),
                    op=OP.mult,
                )
                o64 = spool.tile([P, H1], f32, tag="o64")
                nc.vector.tensor_reduce(out=o64[:], in_=tmp[:],
                                        axis=mybir.AxisListType.X, op=OP.add)
                o64b = spool.tile([P, H1], f32, tag="o64b")
                nc.vector.tensor_tensor(out=o64b[:], in0=o64[:], in1=b1_t[:],
                                        op=OP.add)
                nc.vector.tensor_scalar_max(o64[:], o64b[:], 0.0)
                # h1T[:, k*128:(k+1)*128] = o64.T
                pst = ptpool.tile([H1, P], f32, space="PSUM", tag="pst")
                nc.tensor.transpose(pst[:], o64[:], ident[:])
                nc.vector.tensor_copy(h1T[:, k * P : (k + 1) * P], pst[:])

                # layer-2/3 table rows for this band, then AG the finished
                # quarter so the collective hides behind remaining windows
                ps2 = pspool.tile([P, WB], f32, space="PSUM", tag="xwps")
                nc.tensor.matmul(ps2[:], h1T[:, k * P : (k + 1) * P], w2_t[:],
                                 start=True, stop=True)
                row_t = iopool.tile([P, WB], tdt, tag="rowt")
                nc.vector.tensor_copy(row_t[:], ps2[:])
                nc.vector.tensor_copy(ad23[:, k, 0:5], ps2[:, 325:330])
                nc.vector.tensor_copy(ad23[:, k, 8:13], ps2[:, 335:340])
                nc.sync.dma_start(con2_d[k * P : (k + 1) * P, :], row_t[:])
                if k == NB - 1:
                    nc.gpsimd.collective_compute(
                        "AllGather", mybir.AluOpType.bypass, replica_groups=rg,
                        ins=[con2_d[:]], outs=[tbl2_d[0 : NCORE * MLOC, :]],
                    )

            # ---------------- pass B: layer-2/3 windows ---------------------
            for k in range(NB):
                acc = pspool.tile([P, 336], f32, space="PSUM", tag="acc")
                n_mm = sum(d_c for (_, d_c) in band_chunks[k])
                mm_i = 0
                for ci, (coff, d_c) in enumerate(band_chunks[k]):
                    gt = gpool.tile([P, SLOT_CAP, WB], tdt, tag="gt")
                    nidx = P * d_c
                    nc.gpsimd.dma_gather(
                        gt[:, 0:d_c, :], tbl2_d[:],
                        idx1_t[:, coff : coff + 8 * d_c], nidx, nidx, WB,
                        queue_num=(k * 3 + ci) % 4,
                    )
                    for (asl, adoff, psl) in (
                        (slice(320, 325), 0, slice(320, 325)),
                        (slice(330, 335), 8, slice(330, 335)),
                    ):
                        ut = spool.tile([P, SLOT_CAP, 5], f32, tag="ut")
                        nc.vector.tensor_tensor(
                            out=ut[:, 0:d_c, :], in0=gt[:, 0:d_c, asl],
                            in1=ad23[:, k, adoff : adoff + 5]
                            .unsqueeze(1).to_broadcast([P, d_c, 5]),
                            op=OP.add,
                        )
                        lt = spool.tile([P, SLOT_CAP, 5], f32, tag="lt")
                        nc.scalar.activation(lt[:, 0:d_c, :], ut[:, 0:d_c, :],
                                             AF.Prelu, alpha=NEG)
                        nc.scalar.activation(gt[:, 0:d_c, psl], lt[:, 0:d_c, :],
                                             AF.Exp)
                    for (xsl, psl) in ((slice(0, 160), slice(320, 325)),
                                       (slice(160, 320), slice(330, 335))):
                        nc.vector.tensor_tensor(
                            out=gt[:, 0:d_c, xsl].rearrange(
                                "p d (h c) -> p d h c", h=HEADS),
                            in0=gt[:, 0:d_c, xsl].rearrange(
                                "p d (h c) -> p d h c", h=HEADS),
                            in1=gt[:, 0:d_c, psl].unsqueeze(3).to_broadcast(
                                [P, d_c, 5, H2]),
                            op=OP.mult,
                        )
                    for j in range(d_c):
                        nc.tensor.matmul(acc[:, 0:335], ident_t[:], gt[:, j, 0:335],
                                         start=(mm_i == 0), stop=(mm_i == n_mm - 1))
                        mm_i += 1
                # epilogue: mu and logvar
                for (rsl, dsl, b_t, out_d) in (
                    (slice(0, 160), slice(320, 325), b2_t, mu_d),
                    (slice(160, 320), slice(330, 335), b3_t, lv_d),
                ):
                    den = spool.tile([P, 5], f32, tag="den")
                    nc.vector.tensor_scalar(out=den[:], in0=acc[:, dsl],
                                            scalar1=float(HEADS),
                                            scalar2=HEADS * 1e-16,
                                            op0=OP.mult, op1=OP.add)
                    rec = spool.tile([P, 5], f32, tag="rec")
                    nc.vector.reciprocal(rec[:], den[:])
                    tmp = spool.tile([P, H2, HEADS], f32, tag="tmp2")
                    nc.vector.tensor_tensor(
                        out=tmp[:].transpose([0, 2, 1]),
                        in0=acc[:, rsl].rearrange("p (h c) -> p h c", h=HEADS),
                        in1=rec[:].unsqueeze(2).to_broadcast([P, HEADS, H2]),
                        op=OP.mult,
                    )
                    o32 = spool.tile([P, H2], f32, tag="o32")
                    nc.vector.tensor_reduce(out=o32[:], in_=tmp[:],
                                            axis=mybir.AxisListType.X, op=OP.add)
                    nc.vector.tensor_tensor(out=o32[:], in0=o32[:], in1=b_t[:],
                                            op=OP.add)
                    nc.sync.dma_start(out_d[k * P : (k + 1) * P, :], o32[:])

    nc.compile()
    return nc


# ----------------------------------------------------------------------------
# entry point
# ----------------------------------------------------------------------------
def kernel(x, edge_index, W1, att_src1, att_dst1, b1,
           W2, att_src2, att_dst2, b2,
           W3, att_src3, att_dst3, b3):
    global _compiled
    from concourse.bass_utils import run_bass_kernel_spmd

    x = np.asarray(x, np.float32)
    edge_index = np.asarray(edge_index)

    idx1_all, meta = _prep_graph(edge_index.astype(np.int64))
    chunks, idx_cols = meta["chunks"], meta["idx_cols"]

    key = (tuple(chunks), idx_cols)
    if _compiled is None or _compiled[0] != key:
        nc = _build_program(chunks, idx_cols)
        _compiled = (key, nc)
    nc = _compiled[1]

    # host-side weight augmentation
    w1s, w1dst = _w_aug(np.asarray(W1, np.float32), np.asarray(att_src1),
                        np.asarray(att_dst1), HEADS, H1)
    w1big = np.zeros((FIN, WB), np.float32)
    w1big[:, 0:320] = W1
    w1big[:, 320:325] = w1s
    w1big[:, 325:330] = w1dst

    w2s, w2dst = _w_aug(np.asarray(W2, np.float32), np.asarray(att_src2),
                        np.asarray(att_dst2), HEADS, H2)
    w3s, w3dst = _w_aug(np.asarray(W3, np.float32), np.asarray(att_src3),
                        np.asarray(att_dst3), HEADS, H2)
    w2big = np.zeros((H1, WB), np.float32)
    w2big[:, 0:160] = W2
    w2big[:, 160:320] = W3
    w2big[:, 320:325] = w2s
    w2big[:, 325:330] = w2dst
    w2big[:, 330:335] = w3s
    w2big[:, 335:340] = w3dst

    sent_row = np.zeros((1, WB), np.float16 if TABLE_FP16 else np.float32)
    sent_row[0, 320:340] = -1e4

    core, band, slot = meta["core"], meta["band"], meta["slot"]
    in_maps = []
    for c in range(NCORE):
        m = core == c
        xT = np.zeros((MLOC, FIN), np.float32)
        xT[band[m] * P + slot[m]] = x[m]
        in_maps.append({
            "xT": np.ascontiguousarray(xT.T),
            "w1big": w1big, "w2big": w2big, "sent": sent_row,
            "idx1": np.ascontiguousarray(idx1_all[c]),
            "b1r": np.tile(np.asarray(b1, np.float32)[None, :], (P, 1)),
            "b2r": np.tile(np.asarray(b2, np.float32)[None, :], (P, 1)),
            "b3r": np.tile(np.asarray(b3, np.float32)[None, :], (P, 1)),
        })

    global LAST_RESULTS
    res = run_bass_kernel_spmd(nc, in_maps, core_ids=list(range(NCORE)),
                               trace=TRACE, tmpdir=TRACE_DIR)
    LAST_RESULTS = res

    mu = np.empty((N, H2), np.float32)
    lv = np.empty((N, H2), np.float32)
    rows = band * P + slot
    for c in range(NCORE):
        m = core == c
        mu[m] = res.results[c]["mu"][rows[m]]
        lv[m] = res.results[c]["lv"][rows[m]]
    return mu, mu.copy(), lv
